# revision 46
# baseline (speedup 1.0000x reference)
"""Trainium2 Bass kernel for CIN (Compressed Interaction Network) forward.

Reference computation (per batch b, per dim d, with x = inputs[b, :, d], F=32):
  z0[(h,m)] = x[h]*x[m]                    (1024-vector)
  y0 = relu(W0 @ z0 + b0)                  (128)
  h1 = y0[:64]; f0 = y0[64:]
  z1[(g,m)] = h1[g]*x[m]                   (2048-vector)
  f1 = relu(W1 @ z1 + b1)                  (128)
  out[b, 0:64]  = sum_d f0
  out[b, 64:192] = sum_d f1

Strategy: pure data parallel over 8 cores (256 batch each). Per core the
(b, d) pairs form 16384 GEMM columns, processed as 8 pairs of 1024-column
chunks (DVE ops span a whole 2048-column pair to amortize the ~150ns
per-op overhead).

All z tiles are built with STOCK TENSOR_TENSOR multiplies at DVE 2x.
Layer-0 z (x outer x, symmetrized to 5 row-chunks of 128) multiplies
host-rotated x tiles against the replicated x tile.  Layer-1 z
(y outer x, 16 row-chunks) multiplies per-chunk broadcast-x tiles against
the relu'd y tile; the broadcast tiles come from two sources balanced
across engines: NH1 of them are host-prepared and DMA'd, the remaining
NSEL are produced on-chip by tiny selection matmuls on the Tensor engine
(0/1 stationary, x rows as moving operand) plus a Scalar-engine
PSUM->SBUF copy.  This removes the custom shuffle-multiply DVE op of the
earlier revision, which ran at 1x and dominated the critical path.

The d-reduction (sum of relu'd outputs over the 64 embedding dims) is a
halving tree over the (d-major) free dim on the DVE, pair-merged via 2D
access patterns.
"""

import sys

sys.path.insert(0, "/opt/trn_rl_repo")

import numpy as np

import concourse.bass as bass
import concourse.mybir as mybir
import concourse.tile as tile
from concourse.tile import add_dep_helper
from concourse import bacc
from concourse.bass_utils import run_bass_kernel_spmd

# ---- problem constants (hardcoded per contract) ---------------------------- #
B = 2048
F = 32  # field size (channels in)
D = 64  # embedding dim
O0 = 128  # layer-0 out channels
O1 = 128  # layer-1 out channels
H1 = 64  # split half fed to layer 1
NCORES = 8
BC = B // NCORES  # batch per core
NCHUNK = 1024  # GEMM columns per chunk (16 batch x 64 d)
BPC = NCHUNK // D  # batch elems per chunk
NCHUNKS = BC * D // NCHUNK
NPAIRS = NCHUNKS // 2
W = 2 * NCHUNK  # column width of a chunk pair
L0C = 5  # layer-0 z chunks (symmetric cover: difference classes 0..16)
L0_SHIFT = (0, 4, 8, 12, 16)  # mask shift per layer-0 chunk
L1C = 16  # layer-1 z chunks (2048 rows / 128)
NSEL = 4  # layer-1 chunks built by on-chip selection matmuls
NH1 = L1C - NSEL  # layer-1 chunks with host-prepared broadcast tiles
XLO = 4  # host broadcast tiles double-buffered (c < XLO); rest single-buffered
SEL_CS = tuple(range(NH1, L1C))
MMF = 512  # matmul free-dim per instruction
DT = mybir.dt.float16
FP32 = mybir.dt.float32


# ---- host-side data prep --------------------------------------------------- #
#
# Row maps. Layer 0 exploits z0 symmetry (x[h]*x[m] = x[m]*x[h]): unordered
# pairs {h, m} are classified by circular difference delta = (h-m) mod 32,
# delta in 0..16 (528 pairs total). Chunk c (c in 0..4), quadrant q covers
# class delta = L0_SHIFT[c] + q at all positions:
#   z-row 32q+l of chunk c = x[(l + L0_SHIFT[c] + q) % 32] * x[l]
# The symmetrized weight (W0[o,h,m] + W0[o,m,h]) is assigned to the single
# slot covering {h,m}; duplicate slots (delta 17..19, and half of delta=16)
# carry zero weights.
# Layer 1, chunk c (c in 0..15): z-row 32q+l holds pair
#   (g, m) = (32*(q%2) + l, (c + 16*(q//2)) % 32)
# so the broadcast tile is xb_c[32q+l] = x[(c + 16*(q//2)) % 32] and the y
# tile is y2[32q+l] = y[32*(q%2)+l] (y[0:64] duplicated to partitions 64:128).


def _prep_weights(W0, b0, W1, b1):
    w0 = W0.reshape(O0, F, F)  # [o, h, m]
    w0sym = w0 + w0.transpose(0, 2, 1)
    w0t = np.zeros((L0C, 128, O0), dtype=np.float16)
    for c in range(L0C):
        for q in range(4):
            delta = L0_SHIFT[c] + q
            if delta > 16:
                continue  # duplicate class, keep zero weights
            for l in range(32):
                if delta == 16 and l >= 16:
                    continue  # delta=16 pairs appear twice; keep first half
                h = (l + delta) % 32
                if delta == 0:
                    w0t[c, 32 * q + l, :] = w0[:, l, l].astype(np.float16)
                else:
                    w0t[c, 32 * q + l, :] = w0sym[:, h, l].astype(np.float16)
    w1 = W1.reshape(O1, H1, F)  # [o, g, m]
    w1t = np.empty((L1C, 128, O1), dtype=np.float16)
    for c in range(L1C):
        for q in range(4):
            m = (c + 16 * (q // 2)) % 32
            gbase = 32 * (q % 2)
            w1t[c, 32 * q : 32 * q + 32, :] = w1[:, gbase : gbase + 32, m].T.astype(
                np.float16
            )
    # [p, c, o] layout for contiguous per-partition DMA
    return (
        np.ascontiguousarray(w0t.transpose(1, 0, 2)),
        np.ascontiguousarray(w1t.transpose(1, 0, 2)),
        b0.astype(np.float32),
        b1.astype(np.float32),
    )


def _make_sel():
    """Selection stationaries for the on-chip broadcast matmuls.

    sel[:, 128k + r] with r = 32q+l is 1 exactly at source partition
    (c_k + 16*(q//2)) % 32, so  sel[:, k].T @ x[0:32]  replicates x row
    (c_k + 16*(q//2)) % 32 onto all 32 rows of quadrant q.
    """
    sel = np.zeros((F, NSEL * 128), dtype=np.float16)
    for k, c in enumerate(SEL_CS):
        for q in range(4):
            src = (c + 16 * (q // 2)) % F
            sel[src, 128 * k + 32 * q : 128 * k + 32 * q + 32] = 1.0
    return sel


_I_XA = np.tile(np.arange(F), 4)  # [128] -> l
_I_XR0 = np.stack(
    [
        np.concatenate([(np.arange(F) + L0_SHIFT[c] + q) % F for q in range(4)])
        for c in range(L0C)
    ]
)  # [L0C, 128]
_I_XB = np.stack(
    [
        np.concatenate([np.full(F, (c + 16 * (q // 2)) % F) for q in range(4)])
        for c in range(NH1)
    ]
)  # [NH1, 128]


def _prep_inputs_core(x_core):
    """x_core: (BC, F, D) fp32 -> xa (NPAIRS,128,W), xr0c (NPAIRS,128,L0C,W),
    xbh (NPAIRS,128,NH1,W) fp16."""
    # d-major column order within a chunk (col = d*BPC + b) so the d-sum is
    # a contiguous halving tree over the free dim
    xp = (
        x_core.reshape(NCHUNKS, BPC, F, D)
        .transpose(2, 0, 3, 1)
        .reshape(F, NPAIRS, W)
        .astype(np.float16)
    )
    xa = np.ascontiguousarray(xp[_I_XA].transpose(1, 0, 2))
    xr0c = np.ascontiguousarray(xp[_I_XR0].transpose(2, 1, 0, 3))
    xbh = np.ascontiguousarray(xp[_I_XB].transpose(2, 1, 0, 3))
    return xa, xr0c, xbh


# ---- kernel build ---------------------------------------------------------- #

_NC_CACHE = {}


def _build():
    nc = bacc.Bacc("TRN2", target_bir_lowering=False, debug=False)

    xa_d = nc.dram_tensor("xa", [NPAIRS, 128, W], DT, kind="ExternalInput")
    xr0c_d = nc.dram_tensor("xr0c", [NPAIRS, 128, L0C, W], DT, kind="ExternalInput")
    xbh_d = nc.dram_tensor("xbh", [NPAIRS, 128, NH1, W], DT, kind="ExternalInput")
    sel_d = nc.dram_tensor("sel", [F, NSEL * 128], DT, kind="ExternalInput")
    w0t_d = nc.dram_tensor("w0t", [128, L0C, O0], DT, kind="ExternalInput")
    w1t_d = nc.dram_tensor("w1t", [128, L1C, O1], DT, kind="ExternalInput")
    b0_d = nc.dram_tensor("b0", [O0, 1], FP32, kind="ExternalInput")
    b1_d = nc.dram_tensor("b1", [O1, 1], FP32, kind="ExternalInput")
    out_d = nc.dram_tensor("out", [BC, 192], FP32, kind="ExternalOutput")

    with tile.TileContext(nc) as tc:
        with (
            tc.tile_pool(name="const", bufs=1) as cpool,
            tc.tile_pool(name="xin", bufs=2) as xpool,
            tc.tile_pool(name="xhi", bufs=1) as hpool,
            tc.tile_pool(name="xbs", bufs=2) as spool,
            tc.tile_pool(name="y2p", bufs=2) as ypool,
            tc.tile_pool(name="zp", bufs=3) as zpool,
            tc.tile_pool(name="zp1", bufs=1) as z1pool,
            tc.tile_pool(name="fstage", bufs=2) as fpool,
            tc.tile_pool(name="tstage", bufs=4) as tpool,
            # one shared accumulation pool: per pair the rotation is
            # L0 j0, L0 j1, L1 j0, L1 j1 through two [128,1024] buffers
            tc.tile_pool(name="psacc", bufs=2, space="PSUM") as pspool,
            tc.tile_pool(name="psumb", bufs=2, space="PSUM") as psbpool,
        ):
            # resident weights, biases and selection matrices (DMAs deferred
            # until after the first pair's input tiles so compute starts early)
            w0t = cpool.tile([128, L0C, O0], DT, tag="w0t")
            w1t = cpool.tile([128, L1C, O1], DT, tag="w1t")
            selt = cpool.tile([F, NSEL * 128], DT, tag="sel")
            b0t = cpool.tile([O0, 1], FP32, tag="b0")
            b1t = cpool.tile([O1, 1], FP32, tag="b1")

            def emit_const_dmas():
                nc.sync.dma_start(w0t[:], w0t_d.ap())
                nc.sync.dma_start(w1t[:], w1t_d.ap())
                nc.sync.dma_start(selt[:], sel_d.ap())
                nc.sync.dma_start(b0t[:], b0_d.ap())
                nc.sync.dma_start(b1t[:], b1_d.ap())

            # per-chunk d-sums collect here as [o, b]; transposed at the end
            r0all = cpool.tile([H1, BC], DT, tag="r0all")
            r1all = cpool.tile([128, BC], DT, tag="r1all")

            state = {}  # per-pair live tiles

            def emit_input_dmas(p):
                xa = xpool.tile([128, W], DT, tag="xa")
                xr0 = xpool.tile([128, L0C, W], DT, tag="xr0")
                xbl = xpool.tile([128, XLO, W], DT, tag="xbl")
                xbh = hpool.tile([128, NH1 - XLO, W], DT, tag="xbh")
                dmas = [nc.sync.dma_start(xa[:], xa_d.ap()[p])]
                for c in range(L0C):
                    dmas.append(nc.sync.dma_start(xr0[:, c], xr0c_d.ap()[p, :, c]))
                for k in range(XLO):
                    dmas.append(nc.sync.dma_start(xbl[:, k], xbh_d.ap()[p, :, k]))
                for k in range(XLO, NH1):
                    dmas.append(
                        nc.sync.dma_start(xbh[:, k - XLO], xbh_d.ap()[p, :, k])
                    )
                if p == 1:
                    # pair-1 inputs have no pool WAR yet; without a gate their
                    # transfers steal HBM bandwidth from pair 0's during the
                    # ramp (the 8 DMA queues share it round-robin)
                    for dm in dmas:
                        add_dep_helper(
                            dm.ins, state[0]["last_dma"].ins, sync=True,
                            reason="defer pair-1 inputs behind pair-0",
                        )
                state[p] = {
                    "xa": xa, "xr0": xr0, "xbl": xbl, "xbh": xbh,
                    "last_dma": dmas[-1],
                }

            def emit_select(p, k, j):
                """Selection matmul + PSUM->SBUF copy producing the broadcast
                tile for layer-1 chunk SEL_CS[k], chunk half j, of pair p."""
                st = state[p]
                if k == 0 and j == 0:
                    st["xbs"] = spool.tile(
                        [128, NSEL, W], DT, tag="xbs", name=f"xbs_{p}"
                    )
                xa, xbs = st["xa"], st["xbs"]
                ps = psbpool.tile([128, NCHUNK], FP32, tag="xbp",
                                  name=f"xbp_{p}_{k}_{j}")
                for s in range(NCHUNK // MMF):
                    cols = slice(j * NCHUNK + s * MMF, j * NCHUNK + (s + 1) * MMF)
                    nc.tensor.matmul(
                        ps[:, s * MMF : (s + 1) * MMF],
                        selt[:, k * 128 : (k + 1) * 128],
                        xa[0:F, cols],
                        start=True,
                        stop=True,
                    )
                nc.scalar.activation(
                    xbs[:, k, j * NCHUNK : (j + 1) * NCHUNK],
                    ps[:],
                    mybir.ActivationFunctionType.Copy,
                )

            def _bcast2(ap):
                """[128, W] AP -> [128, 2, W] with a stride-0 middle dim."""
                return ap.rearrange("p (o w) -> p o w", o=1).to_broadcast(
                    [128, 2, W]
                )

            def emit_l0_mul(p, base_c):
                """Layer-0 stock multiplies, two c's fused per DVE op via a
                stride-0 broadcast of xa (c=4 runs alone)."""
                st = state[p]
                nw = min(2, L0C - base_c)
                if nw == 2:
                    z0 = zpool.tile([128, 2, W], DT, tag="z2",
                                    name=f"z0_{p}_{base_c}")
                    nc.vector.tensor_mul(
                        z0[:],
                        st["xr0"][:, base_c : base_c + 2],
                        _bcast2(st["xa"][:]),
                    )
                else:
                    z0 = z1pool.tile([128, 1, W], DT, tag="z",
                                     name=f"z0_{p}_{base_c}")
                    nc.vector.tensor_mul(
                        z0[:, 0], st["xr0"][:, base_c], st["xa"][:]
                    )
                st.setdefault("z0s", []).append((z0, base_c, nw))

            def emit_l0_mms(p):
                """Layer-0 GEMM accumulation for pair p (both chunk halves
                interleaved per c so each z0 tile is released promptly) +
                relus (y2 low half via Scalar, duplicated to the high
                partitions via a DVE copy)."""
                st = state[p]
                z0view = {}
                for z0, base_c, nw in st.pop("z0s"):
                    for dc in range(nw):
                        z0view[base_c + dc] = (z0, dc)
                y2 = ypool.tile([128, W], DT, tag="y2")
                f0s = fpool.tile([H1, 2, NCHUNK], DT, tag="f0s")
                st["y2"], st["f0s"] = y2, f0s
                ps0 = [
                    pspool.tile([128, NCHUNK], FP32, tag="ps", name=f"ps0_{p}_{j}")
                    for j in range(2)
                ]
                for c in range(L0C):
                    z0, dc = z0view[c]
                    for j in range(2):
                        for s in range(NCHUNK // MMF):
                            cols = slice(
                                j * NCHUNK + s * MMF, j * NCHUNK + (s + 1) * MMF
                            )
                            nc.tensor.matmul(
                                ps0[j][:, s * MMF : (s + 1) * MMF],
                                w0t[:, c],
                                z0[:, dc, cols],
                                start=(c == 0),
                                stop=(c == L0C - 1),
                            )
                for j in range(2):
                    jc = slice(j * NCHUNK, (j + 1) * NCHUNK)
                    nc.scalar.activation(
                        y2[:H1, jc],
                        ps0[j][:H1],
                        mybir.ActivationFunctionType.Relu,
                        bias=b0t[:H1],
                    )
                    nc.scalar.activation(
                        f0s[:, j, :],
                        ps0[j][H1:128],
                        mybir.ActivationFunctionType.Relu,
                        bias=b0t[H1:128],
                    )
                    nc.vector.tensor_copy(y2[H1:128, jc], y2[:H1, jc])

            def emit_l1_mul_mm(p, c0):
                """Layer-1 chunks c0, c0+1 for pair p: one fused stock
                multiply (y2 broadcast over the two chunks) feeding the GEMM
                accumulation of both batch-chunks."""
                st = state[p]
                if c0 < XLO:
                    src = st["xbl"][:, c0 : c0 + 2]
                elif c0 < NH1:
                    src = st["xbh"][:, c0 - XLO : c0 - XLO + 2]
                else:
                    src = st["xbs"][:, c0 - NH1 : c0 - NH1 + 2]
                z = zpool.tile([128, 2, W], DT, tag="z2", name=f"z1_{p}_{c0}")
                nc.vector.tensor_mul(z[:], src, _bcast2(st["y2"][:]))
                if c0 == 0:
                    st["ps1"] = [
                        pspool.tile([128, NCHUNK], FP32, tag="ps",
                                    name=f"ps1_{p}_{j}")
                        for j in range(2)
                    ]
                for dc in range(2):
                    c = c0 + dc
                    for j in range(2):
                        for s in range(NCHUNK // MMF):
                            cols = slice(
                                j * NCHUNK + s * MMF, j * NCHUNK + (s + 1) * MMF
                            )
                            nc.tensor.matmul(
                                st["ps1"][j][:, s * MMF : (s + 1) * MMF],
                                w1t[:, c],
                                z[:, dc, cols],
                                start=(c == 0),
                                stop=(c == L1C - 1),
                            )

            def emit_f1_relu(p):
                st = state[p]
                f1s = fpool.tile([128, 2, NCHUNK], DT, tag="f1s")
                st["f1s"] = f1s
                for j in range(2):
                    nc.scalar.activation(
                        f1s[:, j, :],
                        st["ps1"][j][:],
                        mybir.ActivationFunctionType.Relu,
                        bias=b1t[:],
                    )

            def dred_steps(p, which):
                """Pair-merged halving-tree d-reduce micro-steps (closures)."""
                st = state[p]
                fkey, rall = ("f0s", r0all) if which == 0 else ("f1s", r1all)
                fs = st[fkey]

                def make_step(w):
                    def step():
                        nc.vector.tensor_add(
                            fs[:, :, :w], fs[:, :, :w], fs[:, :, w : 2 * w]
                        )

                    return step

                def finish():
                    nc.vector.tensor_add(
                        rall[:, 2 * p * BPC : (2 * p + 2) * BPC].rearrange(
                            "o (j b) -> o j b", j=2
                        ),
                        fs[:, :, :BPC],
                        fs[:, :, BPC : 2 * BPC],
                    )
                    if which == 1:
                        state.pop(p)

                steps = []
                w = NCHUNK // 2
                while w > BPC:
                    steps.append(make_step(w))
                    w //= 2
                steps.append(finish)
                return steps

            # ---- software pipeline ----------------------------------------
            # iteration it: DMA pair it; compute pair cp = it-1; on-chip
            # selects for pair cp+1; d-reduce of pair cp-1 rides the gaps.
            # ---- final transpose (channel, batch) -> (batch, channel) ----
            # seg 0 (batches 0..127 = pairs 0..3) is emitted mid-pipeline so
            # only seg 1 remains in the serial tail
            outbuf = cpool.tile([128, 2, 192], FP32, tag="outbuf")

            def emit_out_seg(seg):
                cs = slice(seg * 128, (seg + 1) * 128)
                pt1 = tpool.tile([128, 128], DT, tag="pt1", name=f"pt1_{seg}")
                nc.sync.dma_start(pt1[:], r1all[:, cs], transpose=True)
                nc.scalar.activation(
                    outbuf[:, seg, H1:192],
                    pt1[:],
                    mybir.ActivationFunctionType.Copy,
                )
                pt0 = tpool.tile([128, H1], DT, tag="pt0", name=f"pt0_{seg}")
                nc.sync.dma_start(pt0[:], r0all[:, cs], transpose=True)
                nc.scalar.activation(
                    outbuf[:, seg, 0:H1],
                    pt0[:],
                    mybir.ActivationFunctionType.Copy,
                )

            pend = []  # pending d-reduce micro-steps

            def drain(n):
                for _ in range(min(n, len(pend))):
                    pend.pop(0)()

            for it in range(NPAIRS + 2):
                if it == 0:
                    emit_const_dmas()
                if it < NPAIRS:
                    emit_input_dmas(it)
                if it == 0:
                    for u in range(2 * NSEL):
                        emit_select(0, u // 2, u % 2)
                    continue
                cp = it - 1
                if cp >= 1:
                    emit_f1_relu(cp - 1)
                if cp == NPAIRS // 2 + 2:
                    emit_out_seg(0)
                if cp < NPAIRS:
                    for base_c in range(0, L0C, 2):
                        emit_l0_mul(cp, base_c)
                    emit_l0_mms(cp)
                    if cp >= 1:
                        pend += dred_steps(cp - 1, 0)
                    for u in range(L1C // 2):
                        emit_l1_mul_mm(cp, 2 * u)
                        if cp + 1 < NPAIRS and u < 2 * NSEL:
                            # spread the next pair's selects through the loop
                            emit_select(cp + 1, u // 2, u % 2)
                        if u == 1 and cp >= 1:
                            pend += dred_steps(cp - 1, 1)
                        drain(2)
                else:
                    # drain the tail
                    pend += dred_steps(cp - 1, 0)
                    pend += dred_steps(cp - 1, 1)
                while pend and cp >= NPAIRS - 1:
                    pend.pop(0)()

            emit_out_seg(1)
            nc.sync.dma_start(
                out_d.ap().rearrange("(s b) c -> b s c", s=2), outbuf[:]
            )

    nc.compile()
    return nc


def _get_nc():
    if "nc" not in _NC_CACHE:
        _NC_CACHE["nc"] = _build()
    return _NC_CACHE["nc"]


def _make_in_maps(inputs, W0, b0, W1, b1):
    w0t, w1t, b0f, b1f = _prep_weights(W0, b0, W1, b1)
    sel = _make_sel()
    in_maps = []
    for core in range(NCORES):
        xa, xr0c, xbh = _prep_inputs_core(inputs[core * BC : (core + 1) * BC])
        in_maps.append(
            {
                "xa": xa,
                "xr0c": xr0c,
                "xbh": xbh,
                "sel": sel,
                "w0t": w0t,
                "w1t": w1t,
                "b0": b0f[:, None],
                "b1": b1f[:, None],
            }
        )
    return in_maps


def kernel(inputs, W0, b0, W1, b1):
    inputs = np.asarray(inputs, dtype=np.float32)
    nc = _get_nc()
    in_maps = _make_in_maps(
        inputs,
        np.asarray(W0, np.float32),
        np.asarray(b0, np.float32),
        np.asarray(W1, np.float32),
        np.asarray(b1, np.float32),
    )
    res = run_bass_kernel_spmd(nc, in_maps, core_ids=list(range(NCORES)))
    out = np.concatenate([res.results[c]["out"] for c in range(NCORES)], axis=0)
    return out.astype(np.float32)


def _install_ntff_hook():
    """The container's antenv lacks axon_hooks; synthesize it around the
    injected libaxon_pjrt.so so run_bass_kernel_spmd(trace=True) works."""
    import types

    if "antenv.axon_hooks" in sys.modules:
        return
    sys.path.insert(0, "/root/.axon_site")
    from trn_agent_boot.trn_boot import _ntff_profile_via_ctypes

    hook = _ntff_profile_via_ctypes("/opt/axon/libaxon_pjrt.so")
    m = types.ModuleType("antenv.axon_hooks")
    m.get_axon_ntff_profile_hook = lambda: hook
    m.set_axon_ntff_profile_hook = lambda h: None
    sys.modules["antenv.axon_hooks"] = m


def profile_once(inputs_dict):
    """Run once with NTFF tracing; return exec_time_ns (core 0)."""
    _install_ntff_hook()
    nc = _get_nc()
    in_maps = _make_in_maps(
        np.asarray(inputs_dict["inputs"], np.float32),
        np.asarray(inputs_dict["W0"], np.float32),
        np.asarray(inputs_dict["b0"], np.float32),
        np.asarray(inputs_dict["W1"], np.float32),
        np.asarray(inputs_dict["b1"], np.float32),
    )
    res = run_bass_kernel_spmd(nc, in_maps, core_ids=list(range(NCORES)), trace=True)
    return res.exec_time_ns


if __name__ == "__main__":
    rng = np.random.default_rng(0)
    inputs = rng.standard_normal((B, F, D), dtype=np.float32)
    W0 = (rng.standard_normal((O0, F * F), dtype=np.float32) * 0.03).astype(np.float32)
    W1 = (rng.standard_normal((O1, H1 * F), dtype=np.float32) * 0.03).astype(np.float32)
    b0 = np.zeros(O0, np.float32)
    b1 = np.zeros(O1, np.float32)
    out = kernel(inputs=inputs, W0=W0, b0=b0, W1=W1, b1=b1)
    print("kernel out", out.shape, out.dtype, out[:2, :4])


# revision 55
# speedup vs baseline: 1.1999x; 1.1999x over previous
"""Trainium2 Bass kernel for CIN (Compressed Interaction Network) forward.

Reference computation (per batch b, per dim d, with x = inputs[b, :, d], F=32):
  z0[(h,m)] = x[h]*x[m]                    (1024-vector)
  y0 = relu(W0 @ z0 + b0)                  (128)
  h1 = y0[:64]; f0 = y0[64:]
  z1[(g,m)] = h1[g]*x[m]                   (2048-vector)
  f1 = relu(W1 @ z1 + b1)                  (128)
  out[b, 0:64]  = sum_d f0
  out[b, 64:192] = sum_d f1

Strategy: pure data parallel over 8 cores (256 batch each). Per core the
(b, d) pairs form 16384 GEMM columns, processed as 8 pairs of 1024-column
chunks (DVE ops span a whole 2048-column pair to amortize the ~150ns
per-op overhead).

All z tiles are built with STOCK TENSOR_TENSOR multiplies at DVE 2x.
Layer-0 z (x outer x, symmetrized to 5 row-chunks of 128) multiplies
host-rotated x tiles against the replicated x tile.  Layer-1 z
(y outer x, 16 row-chunks) multiplies per-chunk broadcast-x tiles against
the relu'd y tile; the broadcast tiles come from two sources balanced
across engines: NH1 of them are host-prepared and DMA'd, the remaining
NSEL are produced on-chip by tiny selection matmuls on the Tensor engine
(0/1 stationary, x rows as moving operand) plus a Scalar-engine
PSUM->SBUF copy.  This removes the custom shuffle-multiply DVE op of the
earlier revision, which ran at 1x and dominated the critical path.

The d-reduction (sum of relu'd outputs over the 64 embedding dims) is a
halving tree over the (d-major) free dim on the DVE, pair-merged via 2D
access patterns.
"""

import sys

sys.path.insert(0, "/opt/trn_rl_repo")

import numpy as np

import concourse.bass as bass
import concourse.mybir as mybir
import concourse.tile as tile
from concourse.tile import add_dep_helper
from concourse import bacc
from concourse.bass_utils import run_bass_kernel_spmd

# ---- problem constants (hardcoded per contract) ---------------------------- #
B = 2048
F = 32  # field size (channels in)
D = 64  # embedding dim
O0 = 128  # layer-0 out channels
O1 = 128  # layer-1 out channels
H1 = 64  # split half fed to layer 1
NCORES = 8
BC = B // NCORES  # batch per core
NCHUNK = 1024  # GEMM columns per chunk (16 batch x 64 d)
BPC = NCHUNK // D  # batch elems per chunk
NCHUNKS = BC * D // NCHUNK
NPAIRS = NCHUNKS // 2
W = 2 * NCHUNK  # column width of a chunk pair
L0C = 5  # layer-0 z chunks (symmetric cover: difference classes 0..16)
L0_SHIFT = (0, 4, 8, 12, 16)  # mask shift per layer-0 chunk
L1C = 16  # layer-1 z chunks (2048 rows / 128)
NSEL = 4  # layer-1 chunks built by on-chip selection matmuls
NH1 = L1C - NSEL  # layer-1 chunks with host-prepared broadcast tiles
XLO = 6  # host broadcast tiles double-buffered (c < XLO); rest single-buffered
SEL_CS = tuple(range(NH1, L1C))
MMF = 512  # matmul free-dim per instruction
DT = mybir.dt.float16
FP32 = mybir.dt.float32


# ---- host-side data prep --------------------------------------------------- #
#
# Row maps. Layer 0 exploits z0 symmetry (x[h]*x[m] = x[m]*x[h]): unordered
# pairs {h, m} are classified by circular difference delta = (h-m) mod 32,
# delta in 0..16 (528 pairs total). Chunk c (c in 0..4), quadrant q covers
# class delta = L0_SHIFT[c] + q at all positions:
#   z-row 32q+l of chunk c = x[(l + L0_SHIFT[c] + q) % 32] * x[l]
# The symmetrized weight (W0[o,h,m] + W0[o,m,h]) is assigned to the single
# slot covering {h,m}; duplicate slots (delta 17..19, and half of delta=16)
# carry zero weights.
# Layer 1, chunk c (c in 0..15): z-row 32q+l holds pair
#   (g, m) = (32*(q%2) + l, (c + 16*(q//2)) % 32)
# so the broadcast tile is xb_c[32q+l] = x[(c + 16*(q//2)) % 32] and the y
# tile is y2[32q+l] = y[32*(q%2)+l] (y[0:64] duplicated to partitions 64:128).


def _prep_weights(W0, b0, W1, b1):
    w0 = W0.reshape(O0, F, F)  # [o, h, m]
    w0sym = w0 + w0.transpose(0, 2, 1)
    w0t = np.zeros((L0C, 128, O0), dtype=np.float16)
    for c in range(L0C):
        for q in range(4):
            delta = L0_SHIFT[c] + q
            if delta > 16:
                continue  # duplicate class, keep zero weights
            for l in range(32):
                if delta == 16 and l >= 16:
                    continue  # delta=16 pairs appear twice; keep first half
                h = (l + delta) % 32
                if delta == 0:
                    w0t[c, 32 * q + l, :] = w0[:, l, l].astype(np.float16)
                else:
                    w0t[c, 32 * q + l, :] = w0sym[:, h, l].astype(np.float16)
    w1 = W1.reshape(O1, H1, F)  # [o, g, m]
    w1t = np.empty((L1C, 128, O1), dtype=np.float16)
    for c in range(L1C):
        for q in range(4):
            m = (c + 16 * (q // 2)) % 32
            gbase = 32 * (q % 2)
            w1t[c, 32 * q : 32 * q + 32, :] = w1[:, gbase : gbase + 32, m].T.astype(
                np.float16
            )
    # [p, c, o] layout for contiguous per-partition DMA
    return (
        np.ascontiguousarray(w0t.transpose(1, 0, 2)),
        np.ascontiguousarray(w1t.transpose(1, 0, 2)),
        b0.astype(np.float32),
        b1.astype(np.float32),
    )


def _make_sel():
    """Selection stationaries for the on-chip broadcast matmuls.

    sel[:, 128k + r] with r = 32q+l is 1 exactly at source partition
    (c_k + 16*(q//2)) % 32, so  sel[:, k].T @ x[0:32]  replicates x row
    (c_k + 16*(q//2)) % 32 onto all 32 rows of quadrant q.
    """
    sel = np.zeros((F, NSEL * 128), dtype=np.float16)
    for k, c in enumerate(SEL_CS):
        for q in range(4):
            src = (c + 16 * (q // 2)) % F
            sel[src, 128 * k + 32 * q : 128 * k + 32 * q + 32] = 1.0
    return sel


_I_XA = np.tile(np.arange(F), 4)  # [128] -> l
_I_XR0 = np.stack(
    [
        np.concatenate([(np.arange(F) + L0_SHIFT[c] + q) % F for q in range(4)])
        for c in range(L0C)
    ]
)  # [L0C, 128]
_I_XB = np.stack(
    [
        np.concatenate([np.full(F, (c + 16 * (q // 2)) % F) for q in range(4)])
        for c in range(NH1)
    ]
)  # [NH1, 128]


def _prep_inputs_core(x_core):
    """x_core: (BC, F, D) fp32 -> xa (NPAIRS,128,W), xr0c (NPAIRS,128,L0C,W),
    xbh (NPAIRS,128,NH1,W) fp16."""
    # d-major column order within a chunk (col = d*BPC + b) so the d-sum is
    # a contiguous halving tree over the free dim
    xp = (
        x_core.reshape(NCHUNKS, BPC, F, D)
        .transpose(2, 0, 3, 1)
        .reshape(F, NPAIRS, W)
        .astype(np.float16)
    )
    xa = np.ascontiguousarray(xp[_I_XA].transpose(1, 0, 2))
    xr0c = np.ascontiguousarray(xp[_I_XR0].transpose(2, 1, 0, 3))
    xbh = np.ascontiguousarray(xp[_I_XB].transpose(2, 1, 0, 3))
    return xa, xr0c, xbh


# ---- kernel build ---------------------------------------------------------- #

_NC_CACHE = {}


def _build():
    nc = bacc.Bacc("TRN2", target_bir_lowering=False, debug=False)

    xa_d = nc.dram_tensor("xa", [NPAIRS, 128, W], DT, kind="ExternalInput")
    xr0c_d = nc.dram_tensor("xr0c", [NPAIRS, 128, L0C, W], DT, kind="ExternalInput")
    xbh_d = nc.dram_tensor("xbh", [NPAIRS, 128, NH1, W], DT, kind="ExternalInput")
    sel_d = nc.dram_tensor("sel", [F, NSEL * 128], DT, kind="ExternalInput")
    w0t_d = nc.dram_tensor("w0t", [128, L0C, O0], DT, kind="ExternalInput")
    w1t_d = nc.dram_tensor("w1t", [128, L1C, O1], DT, kind="ExternalInput")
    b0_d = nc.dram_tensor("b0", [O0, 1], FP32, kind="ExternalInput")
    b1_d = nc.dram_tensor("b1", [O1, 1], FP32, kind="ExternalInput")
    out_d = nc.dram_tensor("out", [BC, 192], FP32, kind="ExternalOutput")

    with tile.TileContext(nc) as tc:
        with (
            tc.tile_pool(name="const", bufs=1) as cpool,
            tc.tile_pool(name="xin", bufs=2) as xpool,
            tc.tile_pool(name="xhi", bufs=1) as hpool,
            tc.tile_pool(name="xbs", bufs=2) as spool,
            tc.tile_pool(name="y2p", bufs=2) as ypool,
            tc.tile_pool(name="zp", bufs=4) as zpool,
            tc.tile_pool(name="fstage", bufs=2) as fpool,
            tc.tile_pool(name="tstage", bufs=4) as tpool,
            # one shared accumulation pool: per pair the rotation is
            # L0 j0, L0 j1, L1 j0, L1 j1 through two [128,1024] buffers
            tc.tile_pool(name="psacc", bufs=2, space="PSUM") as pspool,
            tc.tile_pool(name="psumb", bufs=2, space="PSUM") as psbpool,
        ):
            # resident weights, biases and selection matrices (DMAs deferred
            # until after the first pair's input tiles so compute starts early)
            w0t = cpool.tile([128, L0C, O0], DT, tag="w0t")
            w1t = cpool.tile([128, L1C, O1], DT, tag="w1t")
            selt = cpool.tile([F, NSEL * 128], DT, tag="sel")
            b0t = cpool.tile([O0, 1], FP32, tag="b0")
            b1t = cpool.tile([O1, 1], FP32, tag="b1")

            def emit_const_dmas():
                nc.sync.dma_start(w0t[:], w0t_d.ap())
                nc.sync.dma_start(w1t[:], w1t_d.ap())
                nc.sync.dma_start(selt[:], sel_d.ap())
                nc.sync.dma_start(b0t[:], b0_d.ap())
                nc.sync.dma_start(b1t[:], b1_d.ap())

            # per-chunk d-sums collect here as [o, b]; transposed at the end
            # (r0all lives on partitions 64:128, matching the f0 relu source)
            r0all = cpool.tile([128, BC], DT, tag="r0all")
            r1all = cpool.tile([128, BC], DT, tag="r1all")

            state = {}  # per-pair live tiles

            def emit_input_dmas(p):
                xa = xpool.tile([128, W], DT, tag="xa")
                xr0 = xpool.tile([128, L0C, W], DT, tag="xr0")
                xbl = xpool.tile([128, XLO, W], DT, tag="xbl")
                xbh = hpool.tile([128, NH1 - XLO, W], DT, tag="xbh")
                dmas = [nc.sync.dma_start(xa[:], xa_d.ap()[p])]
                for c in range(L0C):
                    dmas.append(nc.sync.dma_start(xr0[:, c], xr0c_d.ap()[p, :, c]))
                for k in range(XLO):
                    dmas.append(nc.sync.dma_start(xbl[:, k], xbh_d.ap()[p, :, k]))
                for k in range(XLO, NH1):
                    dmas.append(
                        nc.sync.dma_start(xbh[:, k - XLO], xbh_d.ap()[p, :, k])
                    )
                if p == 1:
                    # pair-1 inputs have no pool WAR yet; without a gate their
                    # transfers steal HBM bandwidth from pair 0's during the
                    # ramp (the 8 DMA queues share it round-robin)
                    for dm in dmas:
                        add_dep_helper(
                            dm.ins, state[0]["last_dma"].ins, sync=True,
                            reason="defer pair-1 inputs behind pair-0",
                        )
                state[p] = {
                    "xa": xa, "xr0": xr0, "xbl": xbl, "xbh": xbh,
                    "last_dma": dmas[-1],
                }

            def emit_select(p, k, j):
                """Selection matmul + PSUM->SBUF copy producing the broadcast
                tile for layer-1 chunk SEL_CS[k], chunk half j, of pair p."""
                st = state[p]
                if k == 0 and j == 0:
                    st["xbs"] = spool.tile(
                        [128, NSEL, W], DT, tag="xbs", name=f"xbs_{p}"
                    )
                xa, xbs = st["xa"], st["xbs"]
                ps = psbpool.tile([128, NCHUNK], FP32, tag="xbp",
                                  name=f"xbp_{p}_{k}_{j}")
                for s in range(NCHUNK // MMF):
                    cols = slice(j * NCHUNK + s * MMF, j * NCHUNK + (s + 1) * MMF)
                    nc.tensor.matmul(
                        ps[:, s * MMF : (s + 1) * MMF],
                        selt[:, k * 128 : (k + 1) * 128],
                        xa[0:F, cols],
                        start=True,
                        stop=True,
                    )
                nc.scalar.activation(
                    xbs[:, k, j * NCHUNK : (j + 1) * NCHUNK],
                    ps[:],
                    mybir.ActivationFunctionType.Copy,
                )

            def emit_l0_mul(p, c):
                """One layer-0 pair-merged stock multiply."""
                st = state[p]
                z0 = zpool.tile([128, W], DT, tag="z", name=f"z0_{p}_{c}")
                nc.vector.tensor_mul(z0[:], st["xr0"][:, c], st["xa"][:])
                st.setdefault("z0s", []).append(z0)

            def emit_l0_mms(p):
                """Layer-0 GEMM accumulation for pair p (both chunk halves
                interleaved per c so each z0 tile is released promptly) +
                relus (y2 low half via Scalar, duplicated to the high
                partitions via a DVE copy)."""
                st = state[p]
                z0s = st.pop("z0s")
                y2 = ypool.tile([128, W], DT, tag="y2")
                # fc packs all d-reduce inputs of the pair into one 4-slot
                # tile: slots 0,1 = f1 (all partitions, filled next
                # iteration), slots 2,3 = f0 (partitions 64:128 only; the
                # low half of those slots is never written or read)
                fc = fpool.tile([128, 4, NCHUNK], DT, tag="fc")
                st["y2"], st["fc"] = y2, fc
                ps0 = [
                    pspool.tile([128, NCHUNK], FP32, tag="ps", name=f"ps0_{p}_{j}")
                    for j in range(2)
                ]
                for c in range(L0C):
                    for j in range(2):
                        for s in range(NCHUNK // MMF):
                            cols = slice(
                                j * NCHUNK + s * MMF, j * NCHUNK + (s + 1) * MMF
                            )
                            nc.tensor.matmul(
                                ps0[j][:, s * MMF : (s + 1) * MMF],
                                w0t[:, c],
                                z0s[c][:, cols],
                                start=(c == 0),
                                stop=(c == L0C - 1),
                            )
                for j in range(2):
                    jc = slice(j * NCHUNK, (j + 1) * NCHUNK)
                    nc.scalar.activation(
                        y2[:H1, jc],
                        ps0[j][:H1],
                        mybir.ActivationFunctionType.Relu,
                        bias=b0t[:H1],
                    )
                    nc.scalar.activation(
                        fc[64:128, 2 + j, :],
                        ps0[j][H1:128],
                        mybir.ActivationFunctionType.Relu,
                        bias=b0t[H1:128],
                    )
                    nc.vector.tensor_copy(y2[H1:128, jc], y2[:H1, jc])

            def emit_l1_mul_mm(p, c):
                """One layer-1 chunk c for pair p: pair-merged stock multiply
                against y2, then the GEMM accumulation for both chunks."""
                st = state[p]
                if c < XLO:
                    src = st["xbl"][:, c]
                elif c < NH1:
                    src = st["xbh"][:, c - XLO]
                else:
                    src = st["xbs"][:, c - NH1]
                z = zpool.tile([128, W], DT, tag="z")
                nc.vector.tensor_mul(z[:], src, st["y2"][:])
                if c == 0:
                    st["ps1"] = [
                        pspool.tile([128, NCHUNK], FP32, tag="ps",
                                    name=f"ps1_{p}_{j}")
                        for j in range(2)
                    ]
                for j in range(2):
                    for s in range(NCHUNK // MMF):
                        cols = slice(j * NCHUNK + s * MMF, j * NCHUNK + (s + 1) * MMF)
                        nc.tensor.matmul(
                            st["ps1"][j][:, s * MMF : (s + 1) * MMF],
                            w1t[:, c],
                            z[:, cols],
                            start=(c == 0),
                            stop=(c == L1C - 1),
                        )

            def emit_f1_relu(p):
                st = state[p]
                for j in range(2):
                    nc.scalar.activation(
                        st["fc"][:, j, :],
                        st["ps1"][j][:],
                        mybir.ActivationFunctionType.Relu,
                        bias=b1t[:],
                    )

            def dred_steps(p):
                """Combined halving-tree d-reduce micro-steps for pair p:
                the shared levels process f1 (slots 0,1) and f0 (slots 2,3,
                partitions 64:128 carry data, 0:64 harmless garbage) in one
                [128, 4, w] op; two small finish ops split the results into
                r1all / r0all."""
                fs = state[p]["fc"]
                sl = slice(2 * p * BPC, (2 * p + 2) * BPC)

                def make_step(w):
                    def step():
                        nc.vector.tensor_add(
                            fs[:, :, :w], fs[:, :, :w], fs[:, :, w : 2 * w]
                        )

                    return step

                def finish1():
                    nc.vector.tensor_add(
                        r1all[:, sl].rearrange("o (j b) -> o j b", j=2),
                        fs[:, 0:2, :BPC],
                        fs[:, 0:2, BPC : 2 * BPC],
                    )

                def finish0():
                    nc.vector.tensor_add(
                        r0all[64:128, sl].rearrange("o (j b) -> o j b", j=2),
                        fs[64:128, 2:4, :BPC],
                        fs[64:128, 2:4, BPC : 2 * BPC],
                    )
                    state.pop(p)

                steps = []
                w = NCHUNK // 2
                while w > BPC:
                    steps.append(make_step(w))
                    w //= 2
                steps.append(finish1)
                steps.append(finish0)
                return steps

            # ---- software pipeline ----------------------------------------
            # iteration it: DMA pair it; compute pair cp = it-1; on-chip
            # selects for pair cp+1; d-reduce of pair cp-1 rides the gaps.
            # ---- final transpose (channel, batch) -> (batch, channel) ----
            # seg 0 (batches 0..127 = pairs 0..3) is emitted mid-pipeline so
            # only seg 1 remains in the serial tail
            outbuf = cpool.tile([128, 2, 192], FP32, tag="outbuf")

            def emit_out_seg(seg):
                cs = slice(seg * 128, (seg + 1) * 128)
                pt1 = tpool.tile([128, 128], DT, tag="pt1", name=f"pt1_{seg}")
                nc.sync.dma_start(pt1[:], r1all[:, cs], transpose=True)
                nc.scalar.activation(
                    outbuf[:, seg, H1:192],
                    pt1[:],
                    mybir.ActivationFunctionType.Copy,
                )
                pt0 = tpool.tile([128, H1], DT, tag="pt0", name=f"pt0_{seg}")
                nc.sync.dma_start(pt0[:], r0all[64:128, cs], transpose=True)
                nc.scalar.activation(
                    outbuf[:, seg, 0:H1],
                    pt0[:],
                    mybir.ActivationFunctionType.Copy,
                )

            pend = []  # pending d-reduce micro-steps

            def drain(n):
                for _ in range(min(n, len(pend))):
                    pend.pop(0)()

            for it in range(NPAIRS + 2):
                if it == 0:
                    emit_const_dmas()
                if it < NPAIRS:
                    emit_input_dmas(it)
                if it == 0:
                    for u in range(2 * NSEL):
                        emit_select(0, u // 2, u % 2)
                    continue
                cp = it - 1
                if cp >= 1:
                    emit_f1_relu(cp - 1)
                if cp == NPAIRS // 2 + 2:
                    emit_out_seg(0)
                if cp < NPAIRS:
                    for c in range(L0C):
                        emit_l0_mul(cp, c)
                    emit_l0_mms(cp)
                    if cp >= 1:
                        pend += dred_steps(cp - 1)
                    for c in range(L1C):
                        emit_l1_mul_mm(cp, c)
                        if cp + 1 < NPAIRS and c < 2 * NSEL:
                            # spread the next pair's selects through the loop
                            emit_select(cp + 1, c // 2, c % 2)
                        drain(1)
                else:
                    # drain the tail
                    pend += dred_steps(cp - 1)
                while pend and cp >= NPAIRS - 1:
                    pend.pop(0)()

            emit_out_seg(1)
            nc.sync.dma_start(
                out_d.ap().rearrange("(s b) c -> b s c", s=2), outbuf[:]
            )

    nc.compile()
    return nc


def _get_nc():
    if "nc" not in _NC_CACHE:
        _NC_CACHE["nc"] = _build()
    return _NC_CACHE["nc"]


def _make_in_maps(inputs, W0, b0, W1, b1):
    w0t, w1t, b0f, b1f = _prep_weights(W0, b0, W1, b1)
    sel = _make_sel()
    in_maps = []
    for core in range(NCORES):
        xa, xr0c, xbh = _prep_inputs_core(inputs[core * BC : (core + 1) * BC])
        in_maps.append(
            {
                "xa": xa,
                "xr0c": xr0c,
                "xbh": xbh,
                "sel": sel,
                "w0t": w0t,
                "w1t": w1t,
                "b0": b0f[:, None],
                "b1": b1f[:, None],
            }
        )
    return in_maps


def kernel(inputs, W0, b0, W1, b1):
    inputs = np.asarray(inputs, dtype=np.float32)
    nc = _get_nc()
    in_maps = _make_in_maps(
        inputs,
        np.asarray(W0, np.float32),
        np.asarray(b0, np.float32),
        np.asarray(W1, np.float32),
        np.asarray(b1, np.float32),
    )
    res = run_bass_kernel_spmd(nc, in_maps, core_ids=list(range(NCORES)))
    out = np.concatenate([res.results[c]["out"] for c in range(NCORES)], axis=0)
    return out.astype(np.float32)


def _install_ntff_hook():
    """The container's antenv lacks axon_hooks; synthesize it around the
    injected libaxon_pjrt.so so run_bass_kernel_spmd(trace=True) works."""
    import types

    if "antenv.axon_hooks" in sys.modules:
        return
    sys.path.insert(0, "/root/.axon_site")
    from trn_agent_boot.trn_boot import _ntff_profile_via_ctypes

    hook = _ntff_profile_via_ctypes("/opt/axon/libaxon_pjrt.so")
    m = types.ModuleType("antenv.axon_hooks")
    m.get_axon_ntff_profile_hook = lambda: hook
    m.set_axon_ntff_profile_hook = lambda h: None
    sys.modules["antenv.axon_hooks"] = m


def profile_once(inputs_dict):
    """Run once with NTFF tracing; return exec_time_ns (core 0)."""
    _install_ntff_hook()
    nc = _get_nc()
    in_maps = _make_in_maps(
        np.asarray(inputs_dict["inputs"], np.float32),
        np.asarray(inputs_dict["W0"], np.float32),
        np.asarray(inputs_dict["b0"], np.float32),
        np.asarray(inputs_dict["W1"], np.float32),
        np.asarray(inputs_dict["b1"], np.float32),
    )
    res = run_bass_kernel_spmd(nc, in_maps, core_ids=list(range(NCORES)), trace=True)
    return res.exec_time_ns


if __name__ == "__main__":
    rng = np.random.default_rng(0)
    inputs = rng.standard_normal((B, F, D), dtype=np.float32)
    W0 = (rng.standard_normal((O0, F * F), dtype=np.float32) * 0.03).astype(np.float32)
    W1 = (rng.standard_normal((O1, H1 * F), dtype=np.float32) * 0.03).astype(np.float32)
    b0 = np.zeros(O0, np.float32)
    b1 = np.zeros(O1, np.float32)
    out = kernel(inputs=inputs, W0=W0, b0=b0, W1=W1, b1=b1)
    print("kernel out", out.shape, out.dtype, out[:2, :4])
